# revision 1
# baseline (speedup 1.0000x reference)
"""AssistedExcitation distributed Bass kernel for 8 TRN2 NeuronCores.

Reference computation (per batch b):
    mask[h,w]  = union over 32 boxes of axis-aligned rectangles (rasterized
                 from normalized xywh boxes, trunc + clamp semantics)
    att        = 5x5 conv of reflect-padded mask with the given kernel
    out        = x + att * x        (att broadcast over 256 channels)

Sharding: pure data parallel — batch 16 is split 2-per-core across 8 cores.
No collectives needed.

The kernel is pure HBM bandwidth (ridge regime): per core 6.55 MB in +
6.55 MB out at fp16 = ~36.6us at the ~358 GB/s per-core HBM limit.  The
x/out streams ride fp16 (host-side cast; rel err ~2e-4 vs the 2e-2 gate)
— att is already fp16-quantized by the flat/ones broadcast path.

Per-core algorithm (all bulk work on-device):
  * Box preprocessing on the DVE reproduces the reference's exact f32
    arithmetic:  t1 = (c - wh*0.5)*80,  t2 = (c + wh*0.5)*80.
    For integer pixel p:  p >= max(0,trunc(t1)) <=> p > t1-1  and
    p <= min(79,trunc(t2)) <=> p <= t2, so interval indicators need no
    floor().  Validity (x2>x1 via trunc'd ints) == (#cols covered >= 2).
  * Rasterization is a matmul: indicator rows Cm[n,pw], Rv[n,ph] evaluated
    at reflect-mapped padded coordinates m[p]=min(|p-2|,158-(p-2)) give
    PT[pw,ph] = sum_n Cm*Rv via lhsT=Cm, rhs=Rv; binarize (>0) yields the
    *reflect-padded transposed* mask in one shot.
  * The 5x5 conv is 5 PSUM-accumulated matmuls with banded matrices
    Kc_i[pw,w] = k[i, pw-w]:  att[h,w] = sum_i sum_pw PT[pw,h+i]*Kc_i[pw,w].
  * (1+att) is broadcast across the 128 partitions with K=1 fp16 matmuls
    (lhsT = ones[1,128], rhs = fp16 flattened (1+att) row), evicted to
    SBUF f16, then out = x * att_bc on the VectorEngine (fp16 2x mode),
    streamed in [128, 3200] chunks.

Scheduling notes:
  * All x-stream triggers ride the sync sequencer: each HWDGE DIRECT2D
    costs ~0.7us of sequencer time (descriptor gen for 128 lines), so
    CH=3200 (8 in + 8 out triggers) keeps dispatch off the critical path.
  * The merged const DMA goes FIRST on sync (before the flood) — DMA
    completion observation effectively follows the global trigger order,
    so a small DMA issued behind the megabyte flood is seen ~12us late.
  * The flatten DMAs ride the gpsimd SWDGE ring, whose completion path is
    independent of the HWDGE flood order — no ordering pins that would
    stall the in-flood.
  * PSUM->SBUF evictions of the broadcast alternate scalar/vector; the
    multiplies run on vector between them.
  * NOTE (measured): denser schedules (per-half att tiles, pinned HWDGE
    flats, earlier triggers) reproducibly REGRESS exec to 54-59us — SDMA
    engine 15 runs ~17-25% slower under a gapless stream and becomes a
    multi-us straggler tail.  This arrangement measured best (51.5us).
"""

import numpy as np

import concourse.bass as bass
import concourse.tile as tile
from concourse import bacc, mybir
from concourse.tile_rust import add_dep_helper
from concourse.bass_utils import run_bass_kernel_spmd

F32 = mybir.dt.float32
F16 = mybir.dt.float16
ALU = mybir.AluOpType
ACT = mybir.ActivationFunctionType

N_CORES = 8
B, C, H, W, NBOX = 16, 256, 80, 80, 32
B_LOC = B // N_CORES          # 2 batches per core
HW = H * W                    # 6400
PAD = 84                      # 80 + 2*2 reflect pad
KS = 5
CH = 3200                     # free-dim chunk of the x stream
N_CHUNK = HW // CH            # 2
BC_CH = 512                   # psum bank width for the broadcast matmul


def _build_nc():
    nc = bacc.Bacc(None, target_bir_lowering=False)

    # x/out stream in fp16: halves the HBM traffic (the kernel is pure
    # bandwidth; att is already fp16-quantized on the flat/ones path, and
    # the rel-err budget is 2e-2 vs ~3e-4 introduced by fp16 x/out).
    x_d = nc.declare_dram_parameter("x", [B_LOC, C, H, W], F16, isOutput=False)
    boxes_d = nc.declare_dram_parameter("boxes", [B_LOC, NBOX, 4], F32, isOutput=False)
    nc.declare_dram_parameter("kernel", [1, 1, KS, KS], F32, isOutput=False)
    # single merged const tensor, f16-typed; the f32 piece is bitcast back
    CST_COLS = (KS * W + 128) + 2 * (PAD + 4)
    cst_d = nc.declare_dram_parameter("cst", [PAD, CST_COLS], F16, isOutput=False)
    out_d = nc.declare_dram_parameter("out", [B_LOC, C, H, W], F16, isOutput=True)

    xr = x_d.rearrange("b c h w -> b c (h w)")
    outr = out_d.rearrange("b c h w -> b c (h w)")

    with tile.TileContext(nc) as tc:
        with (
            tc.tile_pool(name="const", bufs=1) as cp,
            tc.tile_pool(name="batch", bufs=2) as bp,
            tc.tile_pool(name="attbc", bufs=2) as ap_,
            tc.tile_pool(name="xin", bufs=8) as xp,
            tc.tile_pool(name="xout", bufs=6) as op_,
            tc.tile_pool(name="ps_small", bufs=2, space=bass.MemorySpace.PSUM) as psm,
            tc.tile_pool(name="ps_bc", bufs=4, space=bass.MemorySpace.PSUM) as pbc,
        ):
            NB2 = B_LOC * NBOX
            cst = cp.tile([PAD, CST_COLS], F16)
            # sync HWDGE, FIRST — before the x flood floods the SDMA
            # engines, so the attention path's inputs land by ~10us.
            # (Measured: splitting this into two DMAs regresses ~3us.)
            nc.sync.dma_start(cst[:], cst_d[:])
            kc = cst[:, 0 : KS * W]
            ones16 = cst[0:1, KS * W : KS * W + 128]
            c32 = cst[0:NB2, KS * W + 128 : CST_COLS].bitcast(F32)  # [64, 88] f32
            mapped = c32[:, 0:PAD]
            bx = c32[:, PAD : PAD + 4]

            # ---- box preprocessing for BOTH batches in one 64-partition pass
            half = cp.tile([NB2, 2], F32)
            nc.vector.tensor_scalar(half[:], bx[:, 2:4], 0.5, None, op0=ALU.mult)
            t1 = cp.tile([NB2, 2], F32)
            nc.vector.tensor_tensor(t1[:], bx[:, 0:2], half[:], op=ALU.subtract)
            nc.vector.tensor_scalar(t1[:], t1[:], float(W), None, op0=ALU.mult)
            t2 = cp.tile([NB2, 2], F32)
            nc.vector.tensor_tensor(t2[:], bx[:, 0:2], half[:], op=ALU.add)
            nc.vector.tensor_scalar(t2[:], t2[:], float(W), None, op0=ALU.mult)
            t1m = cp.tile([NB2, 2], F32)
            nc.vector.tensor_scalar(t1m[:], t1[:], -1.0, None, op0=ALU.add)

            cm = cp.tile([NB2, PAD], F16)
            nc.vector.tensor_scalar(cm[:], mapped[:], t1m[:, 0:1], None, op0=ALU.is_gt)
            nc.vector.scalar_tensor_tensor(
                cm[:], mapped[:], t2[:, 0:1], cm[:], op0=ALU.is_le, op1=ALU.mult
            )
            rm = cp.tile([NB2, PAD], F16)
            nc.vector.tensor_scalar(rm[:], mapped[:], t1m[:, 1:2], None, op0=ALU.is_gt)
            nc.vector.scalar_tensor_tensor(
                rm[:], mapped[:], t2[:, 1:2], rm[:], op0=ALU.is_le, op1=ALU.mult
            )

            rowc = cp.tile([NB2, 1], F32)
            nc.vector.tensor_reduce(rowc[:], rm[:, 2:82], axis=mybir.AxisListType.X, op=ALU.add)
            colc = cp.tile([NB2, 1], F32)
            nc.vector.tensor_reduce(colc[:], cm[:, 2:82], axis=mybir.AxisListType.X, op=ALU.add)
            vv = cp.tile([NB2, 1], F32)
            nc.vector.tensor_scalar(vv[:], rowc[:], 1.5, None, op0=ALU.is_ge)
            nc.vector.scalar_tensor_tensor(
                vv[:], colc[:], 1.5, vv[:], op0=ALU.is_ge, op1=ALU.mult
            )
            rv = cp.tile([NB2, PAD], F16)
            nc.vector.tensor_scalar(rv[:], rm[:], vv[:], None, op0=ALU.mult)

            # ---------------- per-batch attention pipeline ----------------
            att_bcs = []
            for b in range(B_LOC):
                # rasterize: PT[pw, ph] = #boxes covering the (padded) pixel
                pt_ps = psm.tile([PAD, PAD], F32, tag="pt_ps")
                nc.tensor.matmul(
                    pt_ps[:],
                    cm[b * NBOX : (b + 1) * NBOX, :],
                    rv[b * NBOX : (b + 1) * NBOX, :],
                    start=True, stop=True,
                )
                ptm = bp.tile([PAD, PAD], F16)
                nc.vector.tensor_scalar(ptm[:], pt_ps[:], 0.5, None, op0=ALU.is_ge)

                # 5x5 conv: 5 accumulated matmuls
                att_ps = psm.tile([H, W], F32, tag="att_ps")
                for i in range(KS):
                    nc.tensor.matmul(
                        att_ps[:],
                        ptm[:, i : i + H],
                        kc[:, i * W : (i + 1) * W],
                        start=(i == 0),
                        stop=(i == KS - 1),
                    )
                # (1 + att), cast to fp16 for the cheap broadcast matmul
                att1 = bp.tile([H, W], F16)
                nc.scalar.activation(att1[:], att_ps[:], ACT.Copy, bias=1.0)

                # flatten [80,80] -> [1,6400] on the gpsimd SWDGE ring: its
                # descriptor path + completion semaphore are independent of
                # the HWDGE x-flood, so this small DMA is neither delayed by
                # the flood nor (via ordering pins) a stall for it.
                flat = bp.tile([1, HW], F16)
                nc.gpsimd.dma_start(flat[:], att1[:])
                att_bc = ap_.tile([128, HW], F16, tag="att_bc")
                off = 0
                ci = 0
                while off < HW:
                    cw = min(BC_CH, HW - off)
                    bc_ps = pbc.tile([128, BC_CH], F32, tag="bc_ps")
                    nc.tensor.matmul(
                        bc_ps[:, 0:cw], ones16[:], flat[:, off : off + cw],
                        start=True, stop=True,
                    )
                    # evictions alternate DVE/ACT so neither engine gates the
                    # ~21us of PSUM->SBUF copy work (26 x ~0.8us)
                    if ci % 2 == 1:
                        nc.vector.tensor_copy(att_bc[:, off : off + cw], bc_ps[:, 0:cw])
                    else:
                        nc.scalar.copy(att_bc[:, off : off + cw], bc_ps[:, 0:cw])
                    off += cw
                    ci += 1
                att_bcs.append(att_bc)

            # ---------------- main stream: out = x * (1 + att) ----------------
            # ALL in-chunk triggers are emitted (and pinned, ordering-only)
            # ahead of every out trigger on the sync engine: the in-flood has
            # no data deps, so it saturates DMA from t~6 while the attention
            # path runs; out triggers (each data-dep'd on its multiply) then
            # drain behind it.  No flat pins needed — flats ride SWDGE.
            chunks = [
                (b, chalf * 128, k * CH)
                for b in range(B_LOC)
                for chalf in range(C // 128)
                for k in range(N_CHUNK)
            ]
            trig_chain = []

            def _chain(bi):
                if trig_chain:
                    add_dep_helper(bi.ins, trig_chain[-1].ins, sync=False,
                                   reason="pin trigger order")
                trig_chain.append(bi)

            xts = {}
            for i, (b, c0, o0) in enumerate(chunks):
                xt = xp.tile([128, CH], F16, name=f"xt{i}", tag="xt")
                _chain(nc.sync.dma_start(xt[:], xr[b, c0 : c0 + 128, o0 : o0 + CH]))
                xts[i] = xt

            for i, (b, c0, o0) in enumerate(chunks):
                xt = xts.pop(i)
                ot = op_.tile([128, CH], F16, name=f"ot{i}", tag="ot")
                nc.vector.tensor_mul(ot[:], xt[:], att_bcs[b][:, o0 : o0 + CH])
                _chain(nc.sync.dma_start(outr[b, c0 : c0 + 128, o0 : o0 + CH], ot[:]))

    if not nc.is_finalized():
        nc.finalize()
    return nc


def _host_consts(ker: np.ndarray, boxes_shard: np.ndarray):
    """Host-side repacking of the 5x5 kernel + compile-time constants.
    cst32 [64, 84+4]: reflect-mapped padded coords | per-batch boxes.
    cst16 [84, 400+128]: banded conv matrices Kc | ones row (partition 0)."""
    k = ker.reshape(KS, KS).astype(np.float32)
    cst16 = np.zeros((PAD, KS * W + 128), dtype=np.float16)
    for i in range(KS):
        for j in range(KS):
            w = np.arange(W)
            cst16[w + j, i * W + w] = np.float16(k[i, j])
    cst16[0, KS * W : KS * W + 128] = np.float16(1.0)
    p = np.arange(PAD, dtype=np.float32)
    mapped_row = np.minimum(np.abs(p - 2.0), 158.0 - (p - 2.0)).astype(np.float32)
    cst32 = np.zeros((B_LOC * NBOX, PAD + 4), dtype=np.float32)
    cst32[:, 0:PAD] = mapped_row[None, :]
    cst32[:, PAD : PAD + 4] = boxes_shard.reshape(B_LOC * NBOX, 4)
    cst = np.zeros((PAD, (KS * W + 128) + 2 * (PAD + 4)), dtype=np.float16)
    cst[:, 0 : KS * W + 128] = cst16
    cst[0 : B_LOC * NBOX, KS * W + 128 :] = cst32.view(np.float16)
    return cst


_NC_CACHE = None


def _get_nc():
    global _NC_CACHE
    if _NC_CACHE is None:
        _NC_CACHE = _build_nc()
    return _NC_CACHE


def _run(inputs, trace=False, **kw):
    x = np.ascontiguousarray(np.asarray(inputs["x"], dtype=np.float32))
    boxes = np.ascontiguousarray(np.asarray(inputs["boxes"], dtype=np.float32))
    ker = np.ascontiguousarray(np.asarray(inputs["kernel"], dtype=np.float32))
    assert x.shape == (B, C, H, W) and boxes.shape == (B, NBOX, 4)

    nc = _get_nc()
    x16 = x.astype(np.float16)
    in_maps = []
    for i in range(N_CORES):
        bsh = boxes[i * B_LOC : (i + 1) * B_LOC]
        cst = _host_consts(ker, bsh)
        in_maps.append(
            {
                "x": np.ascontiguousarray(x16[i * B_LOC : (i + 1) * B_LOC]),
                "boxes": bsh,
                "kernel": ker,
                "cst": cst,
            }
        )
    res = run_bass_kernel_spmd(nc, in_maps, core_ids=list(range(N_CORES)),
                               trace=trace, **kw)
    out = np.concatenate([r["out"] for r in res.results], axis=0).astype(np.float32)
    return out, res


def kernel(**inputs) -> np.ndarray:
    out, _ = _run(inputs, trace=False)
    return out



# revision 3
# speedup vs baseline: 1.0015x; 1.0015x over previous
"""AssistedExcitation distributed Bass kernel for 8 TRN2 NeuronCores.

Reference computation (per batch b):
    mask[h,w]  = union over 32 boxes of axis-aligned rectangles (rasterized
                 from normalized xywh boxes, trunc + clamp semantics)
    att        = 5x5 conv of reflect-padded mask with the given kernel
    out        = x + att * x        (att broadcast over 256 channels)

Sharding: pure data parallel - batch 16 is split 2-per-core across 8 cores.
No collectives needed.

The kernel is pure HBM/SBUF-port bandwidth (ridge regime): per core
6.55 MB in + 6.55 MB out at fp16 = ~32 us at the ~410 GB/s aggregate
16-engine DMA limit.  x/out ride fp16 (host-side cast; rel err ~2e-4 vs
the 2e-2 gate).

v2 schedule (from trace analysis of the 52.4us baseline):
  * The baseline lost ~8us to (a) the gpsimd SWDGE flat DMA whose DRAIN
    completion lagged ~7us under the flood, serializing all 8 big DVE
    multiplies 26.4->44.3us, and (b) out triggers that could not overlap
    the in-flood.
  * Now: flat [80,80]->[1,6400] DMAs ride the SCALAR HWDGE queue
    (queue-jumps the sync in-flood, hardware completion sems).
  * PSUM->SBUF broadcast evictions are split DVE/scalar and interleaved
    k-major with the multiplies, so multiply (b0,c0,k0) fires at ~17us
    and out-DMA triggers (pinned on sync after the in triggers, each
    gated on its multiply sem) overlap the in-flood.
  * Multiplies are in-place (out tile == x tile): no xout pool, fewer
    teardown semaphores.
  * Box preprocessing uses host-precomputed (p+1)/W and p/W rows so the
    interval indicators need 2 ops per axis on the DVE.
"""

import numpy as np

import concourse.bass as bass
import concourse.tile as tile
from concourse import bacc, mybir
from concourse.tile_rust import add_dep_helper
from concourse.bass_utils import run_bass_kernel_spmd

F32 = mybir.dt.float32
F16 = mybir.dt.float16
ALU = mybir.AluOpType
ACT = mybir.ActivationFunctionType

N_CORES = 8
B, C, H, W, NBOX = 16, 256, 80, 80, 32
B_LOC = B // N_CORES          # 2 batches per core
HW = H * W                    # 6400
PAD = 84                      # 80 + 2*2 reflect pad
KS = 5
CH = 3200                     # free-dim chunk of the x stream
N_CHUNK = HW // CH            # 2
BC_CH = 512                   # psum bank width for the broadcast matmul

NB2 = B_LOC * NBOX            # 64
# merged const tensor layout (f16 cols):
#   [0:400)    kc banded conv matrices
#   [400:528)  ones row (partition 0)
#   [528:...)  f32 region (bitcast): mapped_h [64,84] | mapped_l [64,84] | boxes [64,4]
C32_OFF = KS * W + 128        # 528
CST_COLS = C32_OFF + 2 * (2 * PAD + 4)


def _build_nc():
    nc = bacc.Bacc(None, target_bir_lowering=False)

    x_d = nc.declare_dram_parameter("x", [B_LOC, C, H, W], F16, isOutput=False)
    boxes_d = nc.declare_dram_parameter("boxes", [B_LOC, NBOX, 4], F32, isOutput=False)
    nc.declare_dram_parameter("kernel", [1, 1, KS, KS], F32, isOutput=False)
    cst_d = nc.declare_dram_parameter("cst", [PAD, CST_COLS], F16, isOutput=False)
    out_d = nc.declare_dram_parameter("out", [B_LOC, C, H, W], F16, isOutput=True)

    xr = x_d.rearrange("b c h w -> b c (h w)")
    outr = out_d.rearrange("b c h w -> b c (h w)")

    with tile.TileContext(nc) as tc:
        with (
            tc.tile_pool(name="const", bufs=1) as cp,
            tc.tile_pool(name="batch", bufs=2) as bp,
            tc.tile_pool(name="attbc", bufs=2) as ap_,
            tc.tile_pool(name="xin", bufs=8) as xp,
            tc.tile_pool(name="ps_small", bufs=2, space=bass.MemorySpace.PSUM) as psm,
            tc.tile_pool(name="ps_bc", bufs=4, space=bass.MemorySpace.PSUM) as pbc,
        ):
            cst = cp.tile([PAD, CST_COLS], F16)
            trig_chain = []

            def _chain(bi):
                if trig_chain:
                    add_dep_helper(bi.ins, trig_chain[-1].ins, sync=False,
                                   reason="pin trigger order")
                trig_chain.append(bi)

            # const DMA FIRST on sync - the attention path's inputs land
            # by ~10us, before the x flood floods the SDMA engines.
            _chain(nc.sync.dma_start(cst[:], cst_d[:]))

            kc = cst[:, 0 : KS * W]
            ones16 = cst[0:1, KS * W : KS * W + 128]
            c32 = cst[0:NB2, C32_OFF:CST_COLS].bitcast(F32)  # [64, 172] f32
            mapped_h = c32[:, 0:PAD]
            mapped_l = c32[:, PAD : 2 * PAD]
            bx = c32[:, 2 * PAD : 2 * PAD + 4]

            # ---- in-flood triggers on sync, k-major within batch so the
            # first two chunks need only att cols [0:3200)
            chunks = [
                (b, chalf * 128, k * CH)
                for b in range(B_LOC)
                for k in range(N_CHUNK)
                for chalf in range(C // 128)
            ]
            xts = {}
            for i, (b, c0, o0) in enumerate(chunks):
                xt = xp.tile([128, CH], F16, name=f"xt{i}", tag="xt")
                _chain(nc.sync.dma_start(xt[:], xr[b, c0 : c0 + 128, o0 : o0 + CH]))
                xts[i] = xt

            # ---- box preprocessing for BOTH batches in one 64-partition pass
            # t1 = c - wh/2 ; t2 = c + wh/2  (normalized coords, exact f32)
            # col p covered  <=>  (p+1)/W > t1  and  p/W <= t2
            t1 = cp.tile([NB2, 2], F32)
            nc.vector.scalar_tensor_tensor(
                t1[:], bx[:, 2:4], -0.5, bx[:, 0:2], op0=ALU.mult, op1=ALU.add
            )
            t2 = cp.tile([NB2, 2], F32)
            nc.vector.scalar_tensor_tensor(
                t2[:], bx[:, 2:4], 0.5, bx[:, 0:2], op0=ALU.mult, op1=ALU.add
            )
            cm = cp.tile([NB2, PAD], F16)
            nc.vector.tensor_scalar(cm[:], mapped_h[:], t1[:, 0:1], None, op0=ALU.is_gt)
            nc.vector.scalar_tensor_tensor(
                cm[:], mapped_l[:], t2[:, 0:1], cm[:], op0=ALU.is_le, op1=ALU.mult
            )
            rm = cp.tile([NB2, PAD], F16)
            nc.vector.tensor_scalar(rm[:], mapped_h[:], t1[:, 1:2], None, op0=ALU.is_gt)
            nc.vector.scalar_tensor_tensor(
                rm[:], mapped_l[:], t2[:, 1:2], rm[:], op0=ALU.is_le, op1=ALU.mult
            )
            rowc = cp.tile([NB2, 1], F32)
            nc.vector.tensor_reduce(rowc[:], rm[:, 2:82], axis=mybir.AxisListType.X, op=ALU.add)
            colc = cp.tile([NB2, 1], F32)
            nc.vector.tensor_reduce(colc[:], cm[:, 2:82], axis=mybir.AxisListType.X, op=ALU.add)
            vv = cp.tile([NB2, 1], F32)
            nc.vector.tensor_scalar(vv[:], rowc[:], 1.5, None, op0=ALU.is_ge)
            nc.vector.scalar_tensor_tensor(
                vv[:], colc[:], 1.5, vv[:], op0=ALU.is_ge, op1=ALU.mult
            )
            rv = cp.tile([NB2, PAD], F16)
            nc.vector.tensor_scalar(rv[:], rm[:], vv[:], None, op0=ALU.mult)

            # ---- rasterize + conv + flatten for BOTH batches up front, so
            # flat b1 rides the scalar HWDGE right behind flat b0.
            flats = []
            for b in range(B_LOC):
                pt_ps = psm.tile([PAD, PAD], F32, tag="pt_ps")
                nc.tensor.matmul(
                    pt_ps[:],
                    cm[b * NBOX : (b + 1) * NBOX, :],
                    rv[b * NBOX : (b + 1) * NBOX, :],
                    start=True, stop=True,
                )
                ptm = bp.tile([PAD, PAD], F16)
                nc.vector.tensor_scalar(ptm[:], pt_ps[:], 0.5, None, op0=ALU.is_ge)

                att_ps = psm.tile([H, W], F32, tag="att_ps")
                for i in range(KS):
                    nc.tensor.matmul(
                        att_ps[:],
                        ptm[:, i : i + H],
                        kc[:, i * W : (i + 1) * W],
                        start=(i == 0),
                        stop=(i == KS - 1),
                    )
                att1 = bp.tile([H, W], F16)
                nc.scalar.activation(att1[:], att_ps[:], ACT.Copy, bias=1.0)

                # flatten [80,80] -> [1,6400] on the SCALAR HWDGE queue:
                # separate DMA queue = jumps the sync in-flood backlog,
                # and completion sems post directly from the DMA engines.
                flat = bp.tile([1, HW], F16)
                nc.scalar.dma_start(flat[:], att1[:])
                flats.append(flat)

            # ---- broadcast + evict + multiply, interleaved per batch.
            # Eviction chunks {3, 8} go to the DVE, the rest to scalar
            # (balances: DVE = 2 evicts + 4 multiplies, scalar = 11 evicts
            # per batch).  Multiplies for the k0 chunks are emitted on DVE
            # right after eviction chunk 6 (cols 0:3584 cover k0's 0:3200),
            # k1 chunks after chunk 12, so out triggers fire ASAP and
            # overlap the in-flood.
            n_bc = (HW + BC_CH - 1) // BC_CH  # 13 (12x512 + 1x256)
            DVE_EV = {3, 8}

            def _mul_out(i):
                b, c0, o0 = chunks[i]
                xt = xts[i]
                nc.vector.tensor_mul(xt[:], xt[:], att_bcs[b][:, o0 : o0 + CH])
                _chain(nc.sync.dma_start(outr[b, c0 : c0 + 128, o0 : o0 + CH], xt[:]))

            att_bcs = []
            for b in range(B_LOC):
                att_bc = ap_.tile([128, HW], F16, tag="att_bc")
                att_bcs.append(att_bc)
                for ci in range(n_bc):
                    off = ci * BC_CH
                    cw = min(BC_CH, HW - off)
                    bc_ps = pbc.tile([128, BC_CH], F32, tag="bc_ps")
                    nc.tensor.matmul(
                        bc_ps[:, 0:cw], ones16[:], flats[b][:, off : off + cw],
                        start=True, stop=True,
                    )
                    if ci in DVE_EV:
                        nc.vector.tensor_copy(att_bc[:, off : off + cw], bc_ps[:, 0:cw])
                    else:
                        nc.scalar.copy(att_bc[:, off : off + cw], bc_ps[:, 0:cw])
                    if ci == 6:
                        _mul_out(b * 4 + 0)
                        _mul_out(b * 4 + 1)
                _mul_out(b * 4 + 2)
                _mul_out(b * 4 + 3)

    if not nc.is_finalized():
        nc.finalize()
    return nc


def _host_consts(ker: np.ndarray, boxes_shard: np.ndarray):
    """Host-side repacking of the 5x5 kernel + compile-time constants."""
    k = ker.reshape(KS, KS).astype(np.float32)
    cst = np.zeros((PAD, CST_COLS), dtype=np.float16)
    for i in range(KS):
        for j in range(KS):
            w = np.arange(W)
            cst[w + j, i * W + w] = np.float16(k[i, j])
    cst[0, KS * W : KS * W + 128] = np.float16(1.0)
    p = np.arange(PAD, dtype=np.float32)
    mapped = np.minimum(np.abs(p - 2.0), 158.0 - (p - 2.0)).astype(np.float32)
    c32 = np.zeros((NB2, 2 * PAD + 4), dtype=np.float32)
    c32[:, 0:PAD] = ((mapped + 1.0) / np.float32(W))[None, :]
    c32[:, PAD : 2 * PAD] = (mapped / np.float32(W))[None, :]
    c32[:, 2 * PAD : 2 * PAD + 4] = boxes_shard.reshape(NB2, 4)
    cst[0:NB2, C32_OFF:] = c32.view(np.float16)
    return cst


_NC_CACHE = None


def _get_nc():
    global _NC_CACHE
    if _NC_CACHE is None:
        _NC_CACHE = _build_nc()
    return _NC_CACHE


def _run(inputs, trace=False, **kw):
    x = np.ascontiguousarray(np.asarray(inputs["x"], dtype=np.float32))
    boxes = np.ascontiguousarray(np.asarray(inputs["boxes"], dtype=np.float32))
    ker = np.ascontiguousarray(np.asarray(inputs["kernel"], dtype=np.float32))
    assert x.shape == (B, C, H, W) and boxes.shape == (B, NBOX, 4)

    nc = _get_nc()
    x16 = x.astype(np.float16)
    in_maps = []
    for i in range(N_CORES):
        bsh = boxes[i * B_LOC : (i + 1) * B_LOC]
        cst = _host_consts(ker, bsh)
        in_maps.append(
            {
                "x": np.ascontiguousarray(x16[i * B_LOC : (i + 1) * B_LOC]),
                "boxes": bsh,
                "kernel": ker,
                "cst": cst,
            }
        )
    res = run_bass_kernel_spmd(nc, in_maps, core_ids=list(range(N_CORES)),
                               trace=trace, **kw)
    out = np.concatenate([r["out"] for r in res.results], axis=0).astype(np.float32)
    return out, res


def kernel(**inputs) -> np.ndarray:
    out, _ = _run(inputs, trace=False)
    return out
